# revision 1
# baseline (speedup 1.0000x reference)
"""GQA causal attention (B=2, S=2048, D=4096, H=32, KV=8, HD=128) on 8 TRN2 cores.

Sharding: tensor-parallel over KV-head groups. Each core owns 1 KV head and its
4 query heads: wq/wk/wv column shards, attention for those heads, then an
AllGather of the (transposed) attention outputs followed by a column shard of
the wo projection. Host concatenates the 8 disjoint output column slices.

All matmuls run as float32r (1 cycle/row at moving-dim >= 256). Scores are
computed transposed (scoresT[k, t]) so the softmax reduction over k is done on
the TensorEngine via a ones-vector matmul; exp needs no max-subtraction since
scores ~ N(0, 1) here.
"""

import sys
from contextlib import ExitStack

for _p in ("/opt/trn_rl_repo", "/root/.axon_site/_ro/trn_rl_repo"):
    if _p not in sys.path:
        sys.path.insert(0, _p)

import numpy as np

from concourse import bacc, bass, tile
from concourse.bass_utils import run_bass_kernel_spmd

mybir = bass.mybir
f32 = mybir.dt.float32
f32r = mybir.dt.float32r
AF = mybir.ActivationFunctionType

B, S, D = 2, 2048, 4096
H, KV, HD = 32, 8, 128
NC_ = 8                      # cores
HPC = H // NC_               # 4 q-heads per core
CW = HPC * HD                # 512 attn-output cols per core
T = B * S                    # 4096 tokens
TB = 512                     # token block
NTB = S // TB                # 4 token blocks per batch
NKC = S // 128               # 16 k-chunks per batch
NDC = D // 128               # 32 contraction chunks
SCALE = 1.0 / float(np.sqrt(HD))
RG = [list(range(NC_))]


def _chunked(ap2d):
    """[C*128, N] dram AP -> [128, C, N]."""
    return ap2d.rearrange("(c p) n -> p c n", p=128)


def _projections(nc, G, b):
    """qT (4 heads), kT, v for batch b, including rope and the v transpose."""
    for tb in range(NTB):
        t0 = b * S + tb * TB
        ts_ = slice(tb * TB, (tb + 1) * TB)  # batch-local token slice
        vstage = G["spool"].tile([128, TB], f32r, name="vstage")
        for pas in range(2):
            groups = ["q0", "q1", "q2"] if pas == 0 else ["q3", "k", "v"]
            pss = [G["pproj"].tile([128, TB], f32, name="ps_proj")
                   for _ in groups]
            for dcb in range(8):
                xt4 = G["xtpool"].tile([128, 4, TB], f32r, name="xt4")
                nc.sync.dma_start(
                    xt4[:], _chunked(G["xT"].ap())
                    [:, 4 * dcb:4 * dcb + 4, t0:t0 + TB])
                if pas == 1:
                    wk4 = G["wkvpool"].tile([128, 4, 128], f32r, name="wk4")
                    nc.sync.dma_start(
                        wk4[:], _chunked(G["wk"].ap())[:, 4 * dcb:4 * dcb + 4, :])
                    wv4 = G["wkvpool"].tile([128, 4, 128], f32r, name="wv4")
                    nc.sync.dma_start(
                        wv4[:], _chunked(G["wv"].ap())[:, 4 * dcb:4 * dcb + 4, :])
                for j in range(4):
                    dc = dcb * 4 + j
                    for gi, gname in enumerate(groups):
                        if gname[0] == "q":
                            lhs = G["wq_sb"][int(gname[1])][:, dc, :]
                        elif gname == "k":
                            lhs = wk4[:, j, :]
                        else:
                            lhs = wv4[:, j, :]
                        nc.tensor.matmul(pss[gi][:], lhs, xt4[:, j, :],
                                         start=(dc == 0), stop=(dc == NDC - 1))
            for gi, gname in enumerate(groups):
                if gname[0] == "q":
                    dst = G["qT"][int(gname[1])][:, ts_]
                elif gname == "k":
                    dst = G["kT"][:, ts_]
                else:
                    dst = vstage[:]
                nc.scalar.activation(dst, pss[gi][:], AF.Copy)

        # v: PE-transpose [hd, t] stage into [t, hd] chunks of v_sb
        for jj in range(TB // 128):
            kc = tb * 4 + jj
            pt = G["pproj"].tile([128, TB], f32r, name="ps_vt", tag="ps_proj")
            nc.tensor.transpose(pt[:, 0:128],
                                vstage[:, jj * 128:(jj + 1) * 128],
                                G["eye_sb"][:])
            nc.vector.tensor_copy(G["v_sb"][:, kc * 128:(kc + 1) * 128],
                                  pt[:, 0:128])

        # rope (in place) on the de-interleaved rows of qT / kT:
        #   out = q * [c;c] + swap_halves(q) * [-s;s]
        # (the half-swap crosses partition bases, so it goes through DMA)
        for tgt in [G["qT"][g] for g in range(HPC)] + [G["kT"]]:
            swp = G["tpool"].tile([128, TB], f32r, name="rswp")
            nc.sync.dma_start(swp[0:64, :], tgt[64:128, ts_])
            nc.sync.dma_start(swp[64:128, :], tgt[0:64, ts_])
            t1 = G["tpool"].tile([128, TB], f32r, name="rt1")
            nc.vector.tensor_mul(t1[:], tgt[:, ts_], G["csc_sb"][:, ts_])
            t2 = G["tpool"].tile([128, TB], f32r, name="rt2")
            nc.vector.tensor_mul(t2[:], swp[:], G["css_sb"][:, ts_])
            nc.vector.tensor_add(tgt[:, ts_], t1[:], t2[:])


def _attention(nc, G, b):
    """Causal flash attention for batch b; AllGather per token block."""
    for tau in range(NTB):
        ts_ = slice(tau * TB, (tau + 1) * TB)
        ablk = G["dpool"].tile([CW, TB], f32r, name="ablk")
        for g in range(HPC):
            po = G["pout"].tile([128, TB], f32, name="ps_attn")
            pd = G["pden"].tile([128, TB], f32, name="ps_den")
            nkc = 4 * tau + 4
            for kc in range(nkc):
                psx = G["pscore"].tile([128, TB], f32, name="ps_sc")
                nc.tensor.matmul(psx[:],
                                 G["kT"][:, kc * 128:(kc + 1) * 128],
                                 G["qT"][g][:, ts_],
                                 start=True, stop=True)
                pr = G["ppool"].tile([128, TB], f32r, name="probs")
                nc.scalar.activation(pr[:], psx[:], AF.Exp, scale=SCALE)
                dshift = (kc - 4 * tau) * 128
                if dshift >= 0:
                    nc.vector.tensor_mul(
                        pr[:], pr[:],
                        G["mask_sb"][:, 384 - dshift:896 - dshift])
                nc.tensor.matmul(po[:],
                                 G["v_sb"][:, kc * 128:(kc + 1) * 128],
                                 pr[:],
                                 start=(kc == 0), stop=(kc == nkc - 1))
                nc.tensor.matmul(pd[0:1, :],
                                 G["ones_sb"][:, 0:1], pr[:],
                                 start=(kc == 0), stop=(kc == nkc - 1))
            denr = G["drpool"].tile([1, TB], f32r, name="denr")
            nc.vector.reciprocal(denr[:], pd[0:1, :])
            # broadcast 1/den across partitions: ones[128,1col] x denr[1,TB]
            psb = G["pscore"].tile([128, TB], f32, name="ps_bc", tag="ps_sc")
            nc.tensor.matmul(psb[:], G["onesr_sb"][0:1, :], denr[:],
                             start=True, stop=True)
            denb = G["dbpool"].tile([128, TB], f32, name="denb")
            nc.scalar.activation(denb[:], psb[:], AF.Copy)
            astage = G["spool"].tile([128, TB], f32r, name="astage")
            nc.vector.tensor_mul(astage[:], po[:], denb[:])
            nc.sync.dma_start(ablk[g * 128:(g + 1) * 128, :], astage[:])
        gt = G["gpool"].tile([D, TB], f32r, addr_space="Shared", name="gath_t")
        nc.gpsimd.collective_compute(
            "AllGather", mybir.AluOpType.bypass, replica_groups=RG,
            ins=[ablk.opt()], outs=[gt.opt()])
        G["gathered"].append(gt)


def _wo_phase(nc, tc, G):
    """out[:, CW slice] = attnT_full.T @ wo_shard, streamed per token block."""
    with ExitStack() as st:
        wopool = st.enter_context(tc.tile_pool(name="wo", bufs=1))
        gcpool = st.enter_context(tc.tile_pool(name="gc", bufs=3))
        ospool = st.enter_context(tc.tile_pool(name="ostage", bufs=4))
        pwo = st.enter_context(tc.tile_pool(name="pwo", bufs=8, space="PSUM"))
        wo_sb = wopool.tile([128, NDC, CW], f32r, name="wo_sb")
        nc.sync.dma_start(wo_sb[:], _chunked(G["wo"].ap()))
        for blk in range(B * NTB):
            gt = G["gathered"][blk]
            pts = [pwo.tile([128, CW], f32, name="ps_wo") for _ in range(4)]
            for c in range(NDC):
                gc = gcpool.tile([128, TB], f32r, name="gc")
                nc.sync.dma_start(gc[:], gt[c * 128:(c + 1) * 128, :])
                for tt in range(4):
                    nc.tensor.matmul(pts[tt][:],
                                     gc[:, tt * 128:(tt + 1) * 128],
                                     wo_sb[:, c, :],
                                     start=(c == 0), stop=(c == NDC - 1))
            for tt in range(4):
                ostage = ospool.tile([128, CW], f32, name="ostage")
                nc.vector.tensor_copy(ostage[:], pts[tt][:])
                nc.sync.dma_start(
                    G["out"].ap()[blk * TB + tt * 128:blk * TB + (tt + 1) * 128, :],
                    ostage[:])


def build_graph():
    nc = bacc.Bacc("TRN2", target_bir_lowering=False, debug=False,
                   num_devices=NC_)
    G = {}
    for nm, shape in [("xT", [D, T]), ("wq", [D, CW]), ("wk", [D, HD]),
                      ("wv", [D, HD]), ("wo", [D, CW]), ("csc", [128, S]),
                      ("css", [128, S]),
                      ("maskm", [128, 896]), ("onesv", [128, 1]),
                      ("onesr", [1, 128]), ("eye", [128, 128])]:
        G[nm] = nc.dram_tensor(nm, shape, f32r, kind="ExternalInput")
    G["out"] = nc.dram_tensor("out", [T, CW], f32, kind="ExternalOutput")

    with nc.allow_low_precision(reason="fp32r attention; rel-err gate 2e-2"), \
         tile.TileContext(nc) as tc:
        with ExitStack() as outer:
            G["dpool"] = outer.enter_context(
                tc.tile_pool(name="dram", bufs=3, space="DRAM"))
            G["gpool"] = outer.enter_context(
                tc.tile_pool(name="gath", bufs=8, space="DRAM"))
            G["gathered"] = []

            with ExitStack() as st:
                for nm, kw in [("cpool", dict(name="const", bufs=1)),
                               ("wqpool", dict(name="wqp", bufs=1)),
                               ("qkvpool", dict(name="qkv", bufs=1)),
                               ("xtpool", dict(name="xt", bufs=2)),
                               ("wkvpool", dict(name="wkv", bufs=2)),
                               ("ppool", dict(name="probs", bufs=2)),
                               ("tpool", dict(name="tmp", bufs=3)),
                               ("spool", dict(name="stage", bufs=2)),
                               ("dbpool", dict(name="denb", bufs=2)),
                               ("drpool", dict(name="denr", bufs=2)),
                               ("pproj", dict(name="pproj", bufs=3, space="PSUM")),
                               ("pscore", dict(name="pscore", bufs=2, space="PSUM")),
                               ("pout", dict(name="pout", bufs=2, space="PSUM")),
                               ("pden", dict(name="pden", bufs=1, space="PSUM"))]:
                    G[nm] = st.enter_context(tc.tile_pool(**kw))

                G["csc_sb"] = G["cpool"].tile([128, S], f32r, name="csc_sb")
                nc.sync.dma_start(G["csc_sb"][:], G["csc"][:])
                G["css_sb"] = G["cpool"].tile([128, S], f32r, name="css_sb")
                nc.sync.dma_start(G["css_sb"][:], G["css"][:])
                G["mask_sb"] = G["cpool"].tile([128, 896], f32r, name="mask_sb")
                nc.sync.dma_start(G["mask_sb"][:], G["maskm"][:])
                G["ones_sb"] = G["cpool"].tile([128, 1], f32r, name="ones_sb")
                nc.sync.dma_start(G["ones_sb"][:], G["onesv"][:])
                G["onesr_sb"] = G["cpool"].tile([1, 128], f32r, name="onesr_sb")
                nc.sync.dma_start(G["onesr_sb"][:], G["onesr"][:])
                G["eye_sb"] = G["cpool"].tile([128, 128], f32r, name="eye_sb")
                nc.sync.dma_start(G["eye_sb"][:], G["eye"][:])

                G["wq_sb"] = []
                for g in range(HPC):
                    t_ = G["wqpool"].tile([128, NDC, 128], f32r, name=f"wq_sb{g}")
                    nc.sync.dma_start(
                        t_[:],
                        _chunked(G["wq"].ap())[:, :, g * 128:(g + 1) * 128])
                    G["wq_sb"].append(t_)

                G["qT"] = [G["qkvpool"].tile([128, S], f32r, name=f"qT{g}")
                           for g in range(HPC)]
                G["kT"] = G["qkvpool"].tile([128, S], f32r, name="kT")
                G["v_sb"] = G["qkvpool"].tile([128, S], f32r, name="v_sb")

                for b in range(B):
                    _projections(nc, G, b)
                    _attention(nc, G, b)

            _wo_phase(nc, tc, G)
    nc.compile()
    return nc


_DEINT = np.concatenate([np.arange(0, HD, 2), np.arange(1, HD, 2)])


def _prep_inputs(x, freqs_cos, freqs_sin, wq, wk, wv, wo):
    xT = np.ascontiguousarray(x.reshape(T, D).T.astype(np.float32))
    cT = freqs_cos.T.astype(np.float32)
    sT = freqs_sin.T.astype(np.float32)
    csc = np.ascontiguousarray(np.concatenate([cT, cT], axis=0))
    css = np.ascontiguousarray(np.concatenate([-sT, sT], axis=0))
    jj = np.arange(896)[None, :]
    rr = np.arange(128)[:, None]
    maskm = (rr <= jj - 384).astype(np.float32)
    onesv = np.ones((128, 1), np.float32)
    eye = np.eye(128, dtype=np.float32)

    in_maps = []
    for i in range(NC_):
        qcols = np.concatenate([i * CW + g * HD + _DEINT for g in range(HPC)])
        kcols = i * HD + _DEINT
        vcols = np.arange(i * HD, (i + 1) * HD)
        in_maps.append(dict(
            xT=xT,
            wq=np.ascontiguousarray(wq[:, qcols].astype(np.float32)),
            wk=np.ascontiguousarray(wk[:, kcols].astype(np.float32)),
            wv=np.ascontiguousarray(wv[:, vcols].astype(np.float32)),
            # wo column shard [D, CW]: full attn-dim rows, this core's cols
            wo=np.ascontiguousarray(wo[:, i * CW:(i + 1) * CW].astype(np.float32)),
            csc=csc, css=css, maskm=maskm, onesv=onesv,
            onesr=np.ones((1, 128), np.float32), eye=eye,
        ))
    return in_maps


_CACHE = {}


def _run(inputs, trace=False):
    if "nc" not in _CACHE:
        _CACHE["nc"] = build_graph()
    nc = _CACHE["nc"]
    in_maps = _prep_inputs(
        np.asarray(inputs["x"]), np.asarray(inputs["freqs_cos"]),
        np.asarray(inputs["freqs_sin"]), np.asarray(inputs["wq"]),
        np.asarray(inputs["wk"]), np.asarray(inputs["wv"]),
        np.asarray(inputs["wo"]))
    res = run_bass_kernel_spmd(nc, in_maps, core_ids=list(range(NC_)),
                               trace=trace)
    outs = [res.results[i]["out"] for i in range(NC_)]
    full = np.empty((B, S, D), np.float32)
    for i in range(NC_):
        full[:, :, i * CW:(i + 1) * CW] = outs[i].reshape(B, S, CW)
    return full, res


def kernel(**inputs):
    full, _ = _run(inputs, trace=False)
    return full



# revision 19
# speedup vs baseline: 1.0327x; 1.0327x over previous
"""GQA causal attention (B=2, S=2048, D=4096, H=32, KV=8, HD=128) on 8 TRN2 cores.

Sharding: tensor-parallel over KV-head groups. Each core owns 1 KV head and its
4 query heads: wq/wk/wv column shards, attention for those heads, then an
AllGather of the (transposed) attention outputs followed by a column shard of
the wo projection. Host concatenates the 8 disjoint output column slices.

v2: all matmul operands in bf16 (PSUM accumulation stays f32; rel-err gate is
2e-2 and bf16 lands ~5e-3). This halves x/weight/collective bytes so DMA
(~350us) hides fully under the PE (~800us), which runs near its 1-cycle/row
roofline. Phases are sequential per batch (proj -> attn -> wo) so each fits
the 8 PSUM banks: proj accumulates q0-q3/k/v in 6 banks over a single pass of
x (read once); v is projected directly in [token, hd] orientation (x chunk as
the stationary operand) so no PE-transpose is needed; wo uses 4 accumulators
double-buffered. Scores are computed transposed (scoresT[k, t]) so the softmax
denominator reduces over k on the TensorEngine via a ones-vector matmul; exp
needs no max-subtraction since scores ~ N(0, 1) here.
"""

import sys
from contextlib import ExitStack

for _p in ("/opt/trn_rl_repo", "/root/.axon_site/_ro/trn_rl_repo"):
    if _p not in sys.path:
        sys.path.insert(0, _p)

import ml_dtypes
import numpy as np

from concourse import bacc, bass, tile
from concourse.bass_utils import run_bass_kernel_spmd

mybir = bass.mybir
f32 = mybir.dt.float32
f32r = mybir.dt.float32r
bf16 = mybir.dt.bfloat16
AF = mybir.ActivationFunctionType

B, S, D = 2, 2048, 4096
H, KV, HD = 32, 8, 128
NC_ = 8                      # cores
HPC = H // NC_               # 4 q-heads per core
CW = HPC * HD                # 512 attn-output cols per core
T = B * S                    # 4096 tokens
TB = 512                     # token block
NTB = S // TB                # 4 token blocks per batch
NKC = S // 128               # 16 k-chunks per batch
NDC = D // 128               # 32 contraction chunks
SCALE = 1.0 / float(np.sqrt(HD))
RG = [list(range(NC_))]
SIM = False   # tlprof.py sets True: stub collectives so TimelineSim can run


def _chunked(ap2d):
    """[C*128, N] dram AP -> [128, C, N]."""
    return ap2d.rearrange("(c p) n -> p c n", p=128)


def _proj_phase(nc, G, b, first_rep_pass):
    """qT (4 heads), kT (both [hd, t]) and v ([t, hd] per k-chunk) for batch b,
    including rope.  Single pass over x: 6 concurrent PSUM accumulators.
    On the first pass of a rep, weight/rope-table loads are interleaved with
    the first token block's x loads so the PE starts after ~1 MB of DMA."""
    for tb in range(NTB):
        t0 = b * S + tb * TB
        ts_ = slice(tb * TB, (tb + 1) * TB)  # batch-local token slice
        psq = [G["pproj"].tile([128, TB], f32, name=f"ps_q{g}")
               for g in range(HPC)]
        psk = G["pproj"].tile([128, TB], f32, name="ps_k")
        psv = G["pproj"].tile([128, TB], f32, name="ps_v")
        for dcb in range(8):
            if first_rep_pass and tb == 0:
                cs = slice(4 * dcb, 4 * dcb + 4)
                nc.sync.dma_start(G["wq_sb"][:, cs, :],
                                  _chunked(G["wq"].ap())[:, cs, :])
                nc.sync.dma_start(G["wk_sb"][:, cs, :],
                                  _chunked(G["wk"].ap())[:, cs, :])
                nc.sync.dma_start(G["wv_sb"][:, cs, :],
                                  _chunked(G["wv"].ap())[:, cs, :])
            xt4 = G["xtpool"].tile([128, 4, TB], bf16, name="xt4")
            nc.sync.dma_start(
                xt4[:], _chunked(G["xT"].ap())[:, 4 * dcb:4 * dcb + 4,
                                              t0:t0 + TB])
            for j in range(4):
                dc = dcb * 4 + j
                st_ = (dc == 0)
                sp_ = (dc == NDC - 1)
                for g in range(HPC):
                    nc.tensor.matmul(psq[g][:],
                                     G["wq_sb"][:, dc, g * 128:(g + 1) * 128],
                                     xt4[:, j, :], start=st_, stop=sp_)
                nc.tensor.matmul(psk[:], G["wk_sb"][:, dc, :], xt4[:, j, :],
                                 start=st_, stop=sp_)
                # v in [token, hd] orientation: x chunk is the stationary side.
                # start=True clears has_written for the WHOLE bank, so only
                # the first region's first matmul may carry it; the other tc
                # regions' bits are then unset and flags=0 overwrites+sets.
                for tc in range(4):
                    nc.tensor.matmul(psv[:, tc * 128:(tc + 1) * 128],
                                     xt4[:, j, tc * 128:(tc + 1) * 128],
                                     G["wv_sb"][:, dc, :],
                                     start=(st_ and tc == 0), stop=sp_)
        if first_rep_pass:
            # rope tables for this token block (same for both batches)
            nc.sync.dma_start(G["csc_sb"][:, ts_], G["csc"][:, ts_])
            nc.sync.dma_start(G["css_sb"][:, ts_], G["css"][:, ts_])
        for g in range(HPC):
            nc.any.tensor_copy(G["qT"][g][:, ts_], psq[g][:])
        nc.any.tensor_copy(G["kT"][:, ts_], psk[:])
        nc.any.tensor_copy(G["v_sb"][:, ts_], psv[:])

        # rope (in place) on the de-interleaved rows of qT / kT:
        #   out = q * [c;c] + swap_halves(q) * [-s;s]
        # (the half-swap crosses partition bases, so it goes through DMA)
        for tgt in [G["qT"][g] for g in range(HPC)] + [G["kT"]]:
            swp = G["tpool"].tile([128, TB], bf16, name="rswp")
            nc.sync.dma_start(swp[0:64, :], tgt[64:128, ts_])
            nc.sync.dma_start(swp[64:128, :], tgt[0:64, ts_])
            t1 = G["tpool"].tile([128, TB], bf16, name="rt1")
            nc.vector.tensor_mul(t1[:], tgt[:, ts_], G["csc_sb"][:, ts_])
            t2 = G["tpool"].tile([128, TB], bf16, name="rt2")
            nc.vector.tensor_mul(t2[:], swp[:], G["css_sb"][:, ts_])
            nc.vector.tensor_add(tgt[:, ts_], t1[:], t2[:])


def _attn_phase(nc, G, b):
    """Causal flash attention for batch b; AllGather per token block.

    PE-pipelined: score matmuls run two chunks ahead of the av/den matmuls so
    the PE never waits on the scalar-engine exp; each head's normalization
    tail (reciprocal -> PE broadcast -> divide) is deferred until after the
    NEXT head's chunks are emitted, so the PE broadcast matmul never waits on
    the DVE reciprocal."""
    def emit_tail(t):
        po, pd, ablk_, g_ = t
        denr = G["drpool"].tile([1, TB], f32r, name="denr")
        nc.vector.reciprocal(denr[:], pd[0:1, :])
        # broadcast 1/den across partitions: onesr[1,128] x denr[1,TB]
        psb = G["pscore"].tile([128, TB], f32, name="ps_bc", tag="ps_sc")
        nc.tensor.matmul(psb[:], G["onesr_sb"][0:1, :], denr[:],
                         start=True, stop=True)
        denb = G["dbpool"].tile([128, TB], f32, name="denb")
        nc.any.tensor_copy(denb[:], psb[:])
        astage = G["spool"].tile([128, TB], bf16, name="astage")
        nc.vector.tensor_mul(astage[:], po[:], denb[:])
        nc.sync.dma_start(ablk_[g_ * 128:(g_ + 1) * 128, :], astage[:])

    for tau in range(NTB):
        if tau == 0 and b == 0:
            # wo weights: needed from the wo phase on; DMA is idle during attn
            nc.sync.dma_start(G["wo_sb"][:], _chunked(G["wo"].ap()))
        ts_ = slice(tau * TB, (tau + 1) * TB)
        ablk = G["dpool"].tile([CW, TB], bf16, name="ablk")
        pending = None
        for g in range(HPC):
            po = G["pout"].tile([128, TB], f32, name="ps_attn")
            pd = G["pden"].tile([128, TB], f32, name="ps_den")
            nkc = 4 * tau + 4
            prs = []

            def emit_av(kc, pr):
                # diagonal superblock: tokens < dshift are fully masked, so
                # compute only the unmasked column range [dshift, TB)
                ds = max(0, (kc - 4 * tau) * 128)
                nc.tensor.matmul(po[:, ds:],
                                 G["v_sb"][:, kc * 128:(kc + 1) * 128],
                                 pr[:, ds:],
                                 start=(kc == 0), stop=(kc == nkc - 1))
                nc.tensor.matmul(pd[0:1, ds:],
                                 G["ones_sb"][:, 0:1], pr[:, ds:],
                                 start=(kc == 0), stop=(kc == nkc - 1))

            for kc in range(nkc):
                ds = max(0, (kc - 4 * tau) * 128)
                psx = G["pscore"].tile([128, TB], f32, name="ps_sc")
                nc.tensor.matmul(psx[:, ds:],
                                 G["kT"][:, kc * 128:(kc + 1) * 128],
                                 G["qT"][g][:, tau * TB + ds:(tau + 1) * TB],
                                 start=True, stop=True)
                pr = G["ppool"].tile([128, TB], bf16, name="probs")
                nc.scalar.activation(pr[:, ds:], psx[:, ds:], AF.Exp,
                                     scale=SCALE)
                if kc >= 4 * tau:
                    nc.vector.tensor_mul(
                        pr[:, ds:], pr[:, ds:],
                        G["mask_sb"][:, 384:896 - ds])
                prs.append(pr)
                if kc >= 2:
                    emit_av(kc - 2, prs[kc - 2])
            for kc in range(max(0, nkc - 2), nkc):
                emit_av(kc, prs[kc])
            if pending is not None:
                emit_tail(pending)
            pending = (po, pd, ablk, g)
            if g == HPC - 1:
                # last head's tail must precede this block's AllGather
                emit_tail(pending)
                pending = None
        gt = G["gpool"].tile([D, TB], bf16, addr_space="Shared", name="gath_t")
        if SIM:
            nc.sync.dma_start(gt[0:CW, :], ablk[:])
        else:
            nc.gpsimd.collective_compute(
                "AllGather", mybir.AluOpType.bypass, replica_groups=RG,
                ins=[ablk.opt()], outs=[gt.opt()])
        G["gathered"].append(gt)


def _wo_phase(nc, G, b):
    """out[blk rows, CW slice] = gathered.T @ wo_shard for batch b's blocks."""
    for tau in range(NTB):
        blk = b * NTB + tau
        gt = G["gathered"][blk]
        pts = [G["pwo"].tile([128, CW], f32, name=f"ps_wo{tt}")
               for tt in range(4)]
        for cb in range(NDC // 4):
            # 4 contraction chunks per DMA to amortize descriptor-gen cost
            gc4 = G["gcpool"].tile([128, 4, TB], bf16, name="gc4")
            nc.sync.dma_start(
                gc4[:], _chunked(gt[:])[:, 4 * cb:4 * cb + 4, :])
            for jj in range(4):
                c = 4 * cb + jj
                for tt in range(4):
                    nc.tensor.matmul(pts[tt][:],
                                     gc4[:, jj, tt * 128:(tt + 1) * 128],
                                     G["wo_sb"][:, c, :],
                                     start=(c == 0), stop=(c == NDC - 1))
        for tt in range(4):
            ostage = G["ospool"].tile([128, CW], f32, name="ostage")
            nc.any.tensor_copy(ostage[:], pts[tt][:])
            nc.sync.dma_start(
                G["out"].ap()[blk * TB + tt * 128:blk * TB + (tt + 1) * 128, :],
                ostage[:])


def build_graph(n_repeat=1):
    nc = bacc.Bacc("TRN2", target_bir_lowering=False, debug=False,
                   num_devices=NC_)
    G = {}
    for nm, shape in [("xT", [D, T]), ("wq", [D, CW]), ("wk", [D, HD]),
                      ("wv", [D, HD]), ("wo", [D, CW]), ("csc", [128, S]),
                      ("css", [128, S]), ("maskm", [128, 896]),
                      ("onesv", [128, 1])]:
        G[nm] = nc.dram_tensor(nm, shape, bf16, kind="ExternalInput")
    G["onesr"] = nc.dram_tensor("onesr", [1, 128], f32r, kind="ExternalInput")
    G["out"] = nc.dram_tensor("out", [T, CW], f32, kind="ExternalOutput")

    with nc.allow_low_precision(reason="bf16 attention; rel-err gate 2e-2"), \
         tile.TileContext(nc) as tc:
        with ExitStack() as outer:
            G["dpool"] = outer.enter_context(
                tc.tile_pool(name="dram", bufs=3, space="DRAM"))
            G["gpool"] = outer.enter_context(
                tc.tile_pool(name="gath", bufs=8, space="DRAM"))

            for rep in range(n_repeat):
                G["gathered"] = []
                with ExitStack() as st:
                    for nm, kw in [("cpool", dict(name="const", bufs=1)),
                                   ("wpool", dict(name="wp", bufs=1)),
                                   ("qkvpool", dict(name="qkv", bufs=1)),
                                   ("xtpool", dict(name="xt", bufs=3)),
                                   ("ppool", dict(name="probs", bufs=4)),
                                   ("tpool", dict(name="tmp", bufs=2)),
                                   ("spool", dict(name="stage", bufs=2)),
                                   ("dbpool", dict(name="denb", bufs=2)),
                                   ("drpool", dict(name="denr", bufs=2)),
                                   ("gcpool", dict(name="gc", bufs=3)),
                                   ("ospool", dict(name="ostage", bufs=3))]:
                        kw = dict(kw)
                        kw["name"] = f"{kw['name']}_r{rep}"
                        G[nm] = st.enter_context(tc.tile_pool(**kw))

                    # constants: tiny DMAs, loaded up front; weights and rope
                    # tables load chunk-wise inside the first proj pass
                    G["mask_sb"] = G["cpool"].tile([128, 896], bf16,
                                                   name="mask_sb")
                    nc.sync.dma_start(G["mask_sb"][:], G["maskm"][:])
                    G["ones_sb"] = G["cpool"].tile([128, 1], bf16,
                                                   name="ones_sb")
                    nc.sync.dma_start(G["ones_sb"][:], G["onesv"][:])
                    G["onesr_sb"] = G["cpool"].tile([1, 128], f32r,
                                                    name="onesr_sb")
                    nc.sync.dma_start(G["onesr_sb"][:], G["onesr"][:])

                    G["csc_sb"] = G["cpool"].tile([128, S], bf16, name="csc_sb")
                    G["css_sb"] = G["cpool"].tile([128, S], bf16, name="css_sb")
                    G["wq_sb"] = G["wpool"].tile([128, NDC, CW], bf16,
                                                 name="wq_sb")
                    G["wk_sb"] = G["wpool"].tile([128, NDC, HD], bf16,
                                                 name="wk_sb")
                    G["wv_sb"] = G["wpool"].tile([128, NDC, HD], bf16,
                                                 name="wv_sb")
                    G["wo_sb"] = G["wpool"].tile([128, NDC, CW], bf16,
                                                 name="wo_sb")

                    G["qT"] = [G["qkvpool"].tile([128, S], bf16, name=f"qT{g}")
                               for g in range(HPC)]
                    G["kT"] = G["qkvpool"].tile([128, S], bf16, name="kT")
                    G["v_sb"] = G["qkvpool"].tile([128, S], bf16, name="v_sb")

                    for b in range(B):
                        with tc.tile_pool(name=f"pproj_r{rep}_b{b}", bufs=1,
                                          space="PSUM") as G["pproj"]:
                            _proj_phase(nc, G, b, b == 0)
                        with ExitStack() as ast:
                            G["pscore"] = ast.enter_context(tc.tile_pool(
                                name=f"pscore_r{rep}_b{b}", bufs=3,
                                space="PSUM"))
                            G["pout"] = ast.enter_context(tc.tile_pool(
                                name=f"pout_r{rep}_b{b}", bufs=2,
                                space="PSUM"))
                            G["pden"] = ast.enter_context(tc.tile_pool(
                                name=f"pden_r{rep}_b{b}", bufs=2,
                                space="PSUM"))
                            _attn_phase(nc, G, b)
                        with tc.tile_pool(name=f"pwo_r{rep}_b{b}", bufs=2,
                                          space="PSUM") as G["pwo"]:
                            _wo_phase(nc, G, b)
    nc.compile()
    return nc


_DEINT = np.concatenate([np.arange(0, HD, 2), np.arange(1, HD, 2)])


def _prep_inputs(x, freqs_cos, freqs_sin, wq, wk, wv, wo):
    bf = ml_dtypes.bfloat16
    xT = np.ascontiguousarray(x.reshape(T, D).T.astype(bf))
    cT = freqs_cos.T.astype(np.float32)
    sT = freqs_sin.T.astype(np.float32)
    csc = np.ascontiguousarray(np.concatenate([cT, cT], axis=0).astype(bf))
    css = np.ascontiguousarray(np.concatenate([-sT, sT], axis=0).astype(bf))
    jj = np.arange(896)[None, :]
    rr = np.arange(128)[:, None]
    maskm = (rr <= jj - 384).astype(bf)
    onesv = np.ones((128, 1), bf)

    in_maps = []
    for i in range(NC_):
        qcols = np.concatenate([i * CW + g * HD + _DEINT for g in range(HPC)])
        kcols = i * HD + _DEINT
        vcols = np.arange(i * HD, (i + 1) * HD)
        in_maps.append(dict(
            xT=xT,
            wq=np.ascontiguousarray(wq[:, qcols].astype(bf)),
            wk=np.ascontiguousarray(wk[:, kcols].astype(bf)),
            wv=np.ascontiguousarray(wv[:, vcols].astype(bf)),
            # wo column shard [D, CW]: full attn-dim rows, this core's cols
            wo=np.ascontiguousarray(wo[:, i * CW:(i + 1) * CW].astype(bf)),
            csc=csc, css=css, maskm=maskm, onesv=onesv,
            onesr=np.ones((1, 128), np.float32),
        ))
    return in_maps


_CACHE = {}


def _run(inputs, trace=False):
    if "nc" not in _CACHE:
        _CACHE["nc"] = build_graph()
    nc = _CACHE["nc"]
    in_maps = _prep_inputs(
        np.asarray(inputs["x"]), np.asarray(inputs["freqs_cos"]),
        np.asarray(inputs["freqs_sin"]), np.asarray(inputs["wq"]),
        np.asarray(inputs["wk"]), np.asarray(inputs["wv"]),
        np.asarray(inputs["wo"]))
    res = run_bass_kernel_spmd(nc, in_maps, core_ids=list(range(NC_)),
                               trace=trace)
    outs = [res.results[i]["out"] for i in range(NC_)]
    full = np.empty((B, S, D), np.float32)
    for i in range(NC_):
        full[:, :, i * CW:(i + 1) * CW] = outs[i].reshape(B, S, CW)
    return full, res


def kernel(**inputs):
    full, _ = _run(inputs, trace=False)
    return full
